# revision 11
# baseline (speedup 1.0000x reference)
"""Local (banded) attention, data-parallel over batch on 8 NeuronCores.

The batch (8 elements) is sharded one-per-core via jax.pmap. Each core runs a
block-sparse attention: queries are processed in 128-token blocks and each
block only attends to its 256-key padded window (|i-j| <= 64 band), so score
and softmax work is O(L*256) instead of O(L^2).
"""
import numpy as np
import jax
import jax.numpy as jnp

L, D, H, DH, WIN = 2048, 512, 8, 64, 64
NB = L // 128            # 16 query blocks
_CACHE = {}


def _attn_one(x, wq, wk, wv, bq, bk, bv, wo, bo, masks):
    # x: [L, D]
    q = x @ wq + bq                      # [L, D]
    k = x @ wk + bk
    v = x @ wv + bv
    # pad keys/values by 64 on both ends so every query block sees an aligned
    # 256-wide key window
    kp = jnp.pad(k, ((64, 64), (0, 0)))  # [L+128, D]
    vp = jnp.pad(v, ((64, 64), (0, 0)))
    qb = q.reshape(NB, 128, H, DH).transpose(0, 2, 1, 3)       # [NB,H,128,DH]

    idx = jnp.arange(NB)[:, None] * 128 + jnp.arange(256)[None, :]
    kw = kp[idx].reshape(NB, 256, H, DH).transpose(0, 2, 1, 3)  # [NB,H,256,DH]
    vw = vp[idx].reshape(NB, 256, H, DH).transpose(0, 2, 1, 3)
    s = jnp.einsum('bhqd,bhkd->bhqk', qb, kw) / np.sqrt(DH)     # [NB,H,128,256]
    s = jnp.where(masks[:, None, :, :], s, -jnp.inf)
    p = jax.nn.softmax(s, axis=-1)
    o = jnp.einsum('bhqk,bhkd->bhqd', p, vw)                    # [NB,H,128,DH]
    o = o.transpose(0, 2, 1, 3).reshape(L, D)
    return o @ wo + bo


def _masks():
    r = np.arange(128)[:, None]
    c = np.arange(256)[None, :]
    mid = (c >= r) & (c <= r + 128)
    m = np.broadcast_to(mid, (NB, 128, 256)).copy()
    m[0] &= (c >= 64)
    m[NB - 1] &= (c < 192)
    return jnp.asarray(m)


def kernel(x, in_proj_w, in_proj_b, out_proj_w, out_proj_b):
    x = jnp.asarray(np.asarray(x, dtype=np.float32))
    in_proj_w = np.asarray(in_proj_w)
    in_proj_b = np.asarray(in_proj_b)
    wq = jnp.asarray(in_proj_w[:D].T)
    wk = jnp.asarray(in_proj_w[D:2 * D].T)
    wv = jnp.asarray(in_proj_w[2 * D:].T)
    bq = jnp.asarray(in_proj_b[:D])
    bk = jnp.asarray(in_proj_b[D:2 * D])
    bv = jnp.asarray(in_proj_b[2 * D:])
    wo = jnp.asarray(np.asarray(out_proj_w).T)
    bo = jnp.asarray(np.asarray(out_proj_b))
    masks = _masks()

    if "fn" not in _CACHE:
        _CACHE["fn"] = jax.pmap(_attn_one, in_axes=(0,) + (None,) * 9)
    fn = _CACHE["fn"]
    out = fn(x, wq, wk, wv, bq, bk, bv, wo, bo, masks)
    return np.asarray(out, dtype=np.float32)


# revision 12
# speedup vs baseline: 1.2753x; 1.2753x over previous
"""Local (banded) attention, data-parallel over batch on 8 NeuronCores.

The batch (8 elements) is sharded one-per-core via jax.pmap. Each core runs a
block-sparse attention: queries are processed in 128-token blocks and each
block only attends to its 256-key padded window (|i-j| <= 64 band), so score
and softmax work is O(L*256) instead of O(L^2).
"""
import numpy as np
import jax
import jax.numpy as jnp

L, D, H, DH, WIN = 2048, 512, 8, 64, 64
NB = L // 128            # 16 query blocks
_CACHE = {}


def _attn_one(x, wq, wk, wv, bq, bk, bv, wo, bo, masks):
    # x: [L, D]
    q = x @ wq + bq                      # [L, D]
    k = x @ wk + bk
    v = x @ wv + bv
    # pad keys/values by 64 on both ends so every query block sees an aligned
    # 256-wide key window
    kp = jnp.pad(k, ((64, 64), (0, 0)))  # [L+128, D]
    vp = jnp.pad(v, ((64, 64), (0, 0)))
    qb = q.reshape(NB, 128, H, DH).transpose(0, 2, 1, 3)       # [NB,H,128,DH]

    idx = jnp.arange(NB)[:, None] * 128 + jnp.arange(256)[None, :]
    kw = kp[idx].reshape(NB, 256, H, DH).transpose(0, 2, 1, 3)  # [NB,H,256,DH]
    vw = vp[idx].reshape(NB, 256, H, DH).transpose(0, 2, 1, 3)
    s = jnp.einsum('bhqd,bhkd->bhqk', qb, kw) / np.sqrt(DH)     # [NB,H,128,256]
    s = jnp.where(masks[:, None, :, :], s, -jnp.inf)
    p = jax.nn.softmax(s, axis=-1)
    o = jnp.einsum('bhqk,bhkd->bhqd', p, vw)                    # [NB,H,128,DH]
    o = o.transpose(0, 2, 1, 3).reshape(L, D)
    return o @ wo + bo


def _masks():
    r = np.arange(128)[:, None]
    c = np.arange(256)[None, :]
    mid = (c >= r) & (c <= r + 128)
    m = np.broadcast_to(mid, (NB, 128, 256)).copy()
    m[0] &= (c >= 64)
    m[NB - 1] &= (c < 192)
    return jnp.asarray(m)


def kernel(x, in_proj_w, in_proj_b, out_proj_w, out_proj_b):
    x = jnp.asarray(np.asarray(x, dtype=np.float32))
    in_proj_w = np.asarray(in_proj_w)
    in_proj_b = np.asarray(in_proj_b)
    wq = jnp.asarray(in_proj_w[:D].T)
    wk = jnp.asarray(in_proj_w[D:2 * D].T)
    wv = jnp.asarray(in_proj_w[2 * D:].T)
    bq = jnp.asarray(in_proj_b[:D])
    bk = jnp.asarray(in_proj_b[D:2 * D])
    bv = jnp.asarray(in_proj_b[2 * D:])
    wo = jnp.asarray(np.asarray(out_proj_w).T)
    bo = jnp.asarray(np.asarray(out_proj_b))
    masks = _masks()

    if "fn" not in _CACHE:
        if jax.local_device_count() >= x.shape[0]:
            _CACHE["fn"] = jax.pmap(_attn_one, in_axes=(0,) + (None,) * 9)
        else:
            _CACHE["fn"] = jax.jit(jax.vmap(_attn_one,
                                            in_axes=(0,) + (None,) * 9))
    fn = _CACHE["fn"]
    out = fn(x, wq, wk, wv, bq, bk, bv, wo, bo, masks)
    return np.asarray(out, dtype=np.float32)


# revision 13
# speedup vs baseline: 1.6114x; 1.2636x over previous
"""Local (banded) attention, data-parallel over batch on 8 NeuronCores.

The batch (8 elements) is sharded one-per-core via jax.pmap. Each core runs a
block-sparse attention: queries are processed in 128-token blocks and each
block only attends to its 256-key padded window (|i-j| <= 64 band), so score
and softmax work is O(L*256) instead of O(L^2).
"""
import numpy as np
import jax
import jax.numpy as jnp

L, D, H, DH, WIN = 2048, 512, 8, 64, 64
NB = L // 128            # 16 query blocks
_CACHE = {}


def _attn_one(x, wq, wk, wv, bq, bk, bv, wo, bo, masks):
    # x: [L, D]
    q = x @ wq + bq                      # [L, D]
    k = x @ wk + bk
    v = x @ wv + bv
    # pad keys/values by 64 on both ends so every query block sees an aligned
    # 256-wide key window
    kp = jnp.pad(k, ((64, 64), (0, 0)))  # [L+128, D]
    vp = jnp.pad(v, ((64, 64), (0, 0)))
    qb = q.reshape(NB, 128, H, DH).transpose(0, 2, 1, 3)       # [NB,H,128,DH]

    idx = jnp.arange(NB)[:, None] * 128 + jnp.arange(256)[None, :]
    kw = kp[idx].reshape(NB, 256, H, DH).transpose(0, 2, 1, 3)  # [NB,H,256,DH]
    vw = vp[idx].reshape(NB, 256, H, DH).transpose(0, 2, 1, 3)
    s = jnp.einsum('bhqd,bhkd->bhqk', qb, kw) / np.sqrt(DH)     # [NB,H,128,256]
    s = jnp.where(masks[:, None, :, :], s, -jnp.inf)
    p = jax.nn.softmax(s, axis=-1)
    o = jnp.einsum('bhqk,bhkd->bhqd', p, vw)                    # [NB,H,128,DH]
    o = o.transpose(0, 2, 1, 3).reshape(L, D)
    return o @ wo + bo


def _masks():
    r = np.arange(128)[:, None]
    c = np.arange(256)[None, :]
    mid = (c >= r) & (c <= r + 128)
    m = np.broadcast_to(mid, (NB, 128, 256)).copy()
    m[0] &= (c >= 64)
    m[NB - 1] &= (c < 192)
    return jnp.asarray(m)


def kernel(x, in_proj_w, in_proj_b, out_proj_w, out_proj_b):
    x = jnp.asarray(np.asarray(x, dtype=np.float32))
    in_proj_w = np.asarray(in_proj_w)
    in_proj_b = np.asarray(in_proj_b)
    wq = jnp.asarray(in_proj_w[:D].T)
    wk = jnp.asarray(in_proj_w[D:2 * D].T)
    wv = jnp.asarray(in_proj_w[2 * D:].T)
    bq = jnp.asarray(in_proj_b[:D])
    bk = jnp.asarray(in_proj_b[D:2 * D])
    bv = jnp.asarray(in_proj_b[2 * D:])
    wo = jnp.asarray(np.asarray(out_proj_w).T)
    bo = jnp.asarray(np.asarray(out_proj_b))
    if "masks" not in _CACHE:
        _CACHE["masks"] = _masks()
    masks = _CACHE["masks"]

    if "fn" not in _CACHE:
        if jax.local_device_count() >= x.shape[0]:
            _CACHE["fn"] = jax.pmap(_attn_one, in_axes=(0,) + (None,) * 9)
        else:
            _CACHE["fn"] = jax.jit(jax.vmap(_attn_one,
                                            in_axes=(0,) + (None,) * 9))
    fn = _CACHE["fn"]
    out = fn(x, wq, wk, wv, bq, bk, bv, wo, bo, masks)
    return np.asarray(out, dtype=np.float32)
